# revision 27
# baseline (speedup 1.0000x reference)
"""Trainium2 Bass kernel for segment_reduce (span mean-pool -> entity mean).

Strategy (8 NeuronCores, SPMD, one program + per-core data):
  - Entities are dealt round-robin across the 8 cores in sorted-size order
    ("snake deal"), so every core owns e_pc entities whose per-rank row
    counts nearly match across cores.  Each core owns all mentions of its
    entities, so no cross-core reduction is needed.
  - The host lays each core's span token-rows out in *streaming order*:
    rank-k entity occupies padded slot range [S[k], S[k+1]) identical on
    every core (quota = max rows over cores).  The table is uploaded as
    [128, SLOTS*D] where slot i = (chunk i//128, part i%128), so the device
    needs NO gather at all - just big sequential direct DMAs.
  - Rows are split into two precision classes: mentions shorter than
    LEN_SPLIT tokens (large 1/len weights) stay fp16; longer mentions are
    stored as float8e3 (e3m4).  Each class has its own slot schedule/table;
    both feed the same PSUM accumulators (measured rel err ~9e-3 vs fp32).
  - Because slots are entity-sorted, each 128-row chunk covers only a few
    consecutive entity ranks.  The span-sum AND entity-scatter fuse into a
    single narrow PSUM-accumulated matmul per chunk:
        psum[rank, :] += sum_p W[p, rank] * row[p, :]
    where W[p, col] (fp16, one 64-wide zero-padded block per matmul, base
    partition must be in {0,32,64}) is pure index data, built on the host
    and uploaded ONCE (not per iteration).
  - Per iteration the device does: ~12 direct DMAs (~5.8MB), ~183 narrow
    matmuls (PE), 4 psum->sbuf fp16 copies, 1 fp16 output DMA (host
    upcasts).  Vector/Pool engines are essentially idle; the kernel runs at
    the joint HBM-read / PE-stream roofline (~15us/iter vs 78us for the
    indirect-DMA + on-device-W baseline).
"""

import contextlib

import numpy as np

from concourse import bass, mybir
import concourse.tile as tile
from concourse.bass_utils import run_bass_kernel_spmd

# Problem constants (nn_BaseModel_69355131896059)
T, D, M, E, L_MAX = 200000, 256, 20000, 4000, 16
N_CORES = 8
FP32 = mybir.dt.float32
FP16 = mybir.dt.float16
FP8E3 = mybir.dt.float8e3

LEN_SPLIT = 4   # mentions shorter than this stay fp16; longer go fp8 (e3m4)
SEG_SLOTS = 16  # 128-row chunks per DMA segment
WIN = 64        # psum window width (base partition must be in {0,32,64})


def _np_dt(dt):
    return mybir.dt.np(dt)


# ---------------------------------------------------------------------------
# Walrus in this container rejects instructions carrying more than ~2 sync
# commands ("Too many sync wait commands").  After Tile scheduling, split
# excess sem waits onto same-engine NOPs inserted before the instruction.
# ---------------------------------------------------------------------------
_WAIT_LIMIT = 1
_nsplit = [0]


def split_excess_waits(nc, limit=_WAIT_LIMIT):
    for fn in nc.m.functions:
        for bb in fn.blocks:
            insts = list(bb.instructions)
            if not any(
                i.sync_info is not None
                and i.sync_info.on_wait
                and len(i.sync_info.on_wait) > limit
                for i in insts
            ):
                continue
            out = []
            for inst in insts:
                si = inst.sync_info
                if si is not None and si.on_wait and len(si.on_wait) > limit:
                    waits = list(si.on_wait)
                    keep, extra = waits[-limit:], waits[:-limit]
                    for s in range(0, len(extra), limit):
                        nop = mybir.InstNoOp(
                            name=f"waitsplit-{_nsplit[0]}",
                            engine=inst.engine,
                            sync_info=mybir.SyncInfo(
                                on_wait=extra[s : s + limit], on_update=[]
                            ),
                        )
                        _nsplit[0] += 1
                        out.append(nop)
                    inst.sync_info = mybir.SyncInfo(
                        on_wait=keep, on_update=list(si.on_update or [])
                    )
                out.append(inst)
            bb.instructions = out


# ---------------------------------------------------------------------------
# Host-side prep: entity->core snake deal, per-class slot schedules, W
# matrix, tables.  All of it is index bookkeeping + byte movement; the
# arithmetic (sums, weighting) happens on device.
# ---------------------------------------------------------------------------
def _host_prep(info, num_entities):
    E_ = int(num_entities)
    eid = np.asarray(info[:, 0], dtype=np.int64)
    starts = np.asarray(info[:, 2], dtype=np.int64)
    ends = np.asarray(info[:, 3], dtype=np.int64)
    lens = ends - starts
    glen = np.minimum(lens, L_MAX)  # reference only pools the first L_MAX rows
    cnt = np.bincount(eid, minlength=E_).astype(np.float64)
    w_all = 1.0 / (np.maximum(lens, 1) * np.maximum(cnt[eid], 1.0))

    cls_of_m = (lens >= LEN_SPLIT).astype(np.int64)  # 0 = fp16, 1 = fp8
    n_cls = 2
    bytes_of_cls = np.array([2, 1])

    # Deal entities to cores: sort by fp8 rows (the dominant class) so each
    # rank group of 8 has near-equal fp8 quota, then re-sort within blocks of
    # 128 by fp16 rows to also equalize the fp16 quota (r8 spread within a
    # block is tiny, so this costs the fp8 class almost nothing).
    ent_r = np.zeros((E_, n_cls), dtype=np.int64)
    np.add.at(ent_r, (eid, cls_of_m), glen)
    e_pc = -(-E_ // N_CORES)
    order = np.argsort(-ent_r[:, 1], kind="stable")
    BLK = 128
    for i in range(0, E_, BLK):
        blk = order[i : i + BLK]
        order[i : i + BLK] = blk[np.argsort(-ent_r[blk, 0], kind="stable")]
    order_pad = np.concatenate([order, np.full(e_pc * N_CORES - E_, -1, np.int64)])
    groups = order_pad.reshape(e_pc, N_CORES)  # [rank, core] -> entity (-1 pad)
    rank_of_ent = np.zeros(E_, dtype=np.int64)
    core_of_ent = np.zeros(E_, dtype=np.int64)
    rk, ck = np.nonzero(groups >= 0)
    rank_of_ent[groups[rk, ck]] = rk
    core_of_ent[groups[rk, ck]] = ck
    m_rank = rank_of_ent[eid]
    m_core = core_of_ent[eid]

    n_etiles = -(-e_pc // 128)
    classes = []
    n_ops_so_far = 0
    mm_ops = []  # (class, chunk, psum_tile, psum_part_off, wcol_a)
    for cl in range(n_cls):
        msel = cls_of_m == cl
        # per-(rank, core) class rows -> quota
        per_rc = np.zeros((e_pc, N_CORES), dtype=np.int64)
        np.add.at(per_rc, (m_rank[msel], m_core[msel]), glen[msel])
        quota = per_rc.max(axis=1)
        S = np.concatenate([[0], np.cumsum(quota)])
        n_slots = int(S[-1])
        n_chunks = max(1, -(-n_slots // 128))
        padded = n_chunks * 128

        c_start = np.arange(n_chunks) * 128
        l0 = np.clip(np.searchsorted(S, c_start, side="right") - 1, 0, e_pc - 1)
        l1 = np.minimum(np.searchsorted(S, c_start + 128, side="left"), e_pc)
        l1 = np.maximum(l1, l0 + 1)
        a0 = l0 // WIN
        a1 = (l1 - 1) // WIN
        op_base = n_ops_so_far + np.concatenate(
            [[0], np.cumsum(a1 - a0 + 1)]
        ).astype(np.int64)
        for c in range(n_chunks):
            for a in range(int(a0[c]), int(a1[c]) + 1):
                t = (WIN * a) // 128
                mm_ops.append((cl, c, t, WIN * a - 128 * t, WIN * len(mm_ops)))
        n_ops_so_far = len(mm_ops)

        rank_of_slot = np.clip(
            np.searchsorted(S, np.arange(padded), side="right") - 1, 0, e_pc - 1
        )
        chunk_of_slot = np.arange(padded) // 128
        wcol_of_slot = (
            WIN * (op_base[chunk_of_slot] + rank_of_slot // WIN - a0[chunk_of_slot])
            + rank_of_slot % WIN
        )
        classes.append(dict(
            msel=msel, S=S, n_chunks=n_chunks, padded=padded,
            wcol_of_slot=wcol_of_slot,
        ))

    w_cols = WIN * len(mm_ops)
    first_of, last_of = {}, {}
    for i, (cl, c, t, o, wa) in enumerate(mm_ops):
        first_of.setdefault((t, o), i)
        last_of[(t, o)] = i

    # per-core slot -> (enc row | -1, weight); shared-structure W matrix
    wgt_mat = np.zeros((N_CORES, 128, w_cols), dtype=np.float32)
    rows_of_slot = []
    for cl in range(n_cls):
        k = classes[cl]
        row_of_slot = np.full((N_CORES, k["padded"]), -1, dtype=np.int64)
        for c in range(N_CORES):
            sel = np.nonzero((m_core == c) & k["msel"])[0]
            sel = sel[np.argsort(m_rank[sel], kind="stable")]
            g = glen[sel]
            cum = np.cumsum(g) - g
            rank_base = np.cumsum(np.bincount(
                m_rank[sel], weights=g.astype(np.float64), minlength=e_pc))
            rank_base = np.concatenate([[0], rank_base])[:-1].astype(np.int64)
            base = k["S"][m_rank[sel]] + (cum - rank_base[m_rank[sel]])
            within = np.arange(int(g.sum())) - np.repeat(cum, g)
            tok_slot = (np.repeat(base, g) + within).astype(np.int64)
            tok_row = np.repeat(starts[sel], g) + within
            tok_w = np.repeat(w_all[sel], g)
            row_of_slot[c, tok_slot] = tok_row
            wgt_mat[c, tok_slot % 128, k["wcol_of_slot"][tok_slot]] = tok_w
        rows_of_slot.append(row_of_slot)

    ent_of_core = [groups[:, c][groups[:, c] >= 0] for c in range(N_CORES)]
    return {
        "n_chunks": [classes[cl]["n_chunks"] for cl in range(n_cls)],
        "n_etiles": n_etiles,
        "w_cols": w_cols,
        "mm_ops": mm_ops,
        "first_of": first_of,
        "last_of": last_of,
        "rows_of_slot": rows_of_slot,
        "wgt_mat": wgt_mat,
        "ent_of_core": ent_of_core,
        "e_pc": e_pc,
        "E": E_,
    }


CLS_DT = [FP16, FP8E3]


def build_tables(enc_np, prep):
    """Per-core, per-class streaming-order tables [128, SLOTS*D]."""
    tabs = [[] for _ in range(2)]
    for cl in range(2):
        n_chunks = prep["n_chunks"][cl]
        padded = n_chunks * 128
        ndt = _np_dt(CLS_DT[cl])
        for c in range(N_CORES):
            rows = prep["rows_of_slot"][cl][c]
            tab = np.zeros((padded, D), dtype=ndt)
            v = rows >= 0
            tab[v] = enc_np[rows[v]].astype(ndt)
            tab = np.ascontiguousarray(
                tab.reshape(n_chunks, 128, D).transpose(1, 0, 2)
                .reshape(128, n_chunks * D)
            )
            tabs[cl].append(tab)
    return tabs


# ---------------------------------------------------------------------------
# Device program
# ---------------------------------------------------------------------------
def build_program(prep, n_reps=1, mode="full", seg_slots=SEG_SLOTS, gat_slack=2):
    n_etiles = prep["n_etiles"]
    w_cols = prep["w_cols"]
    mm_ops = prep["mm_ops"]
    first_of = prep["first_of"]
    last_of = prep["last_of"]

    nc = bass.Bass("TRN2", target_bir_lowering=False, debug=False,
                   num_devices=N_CORES)
    encs = [
        nc.dram_tensor(f"enc{cl}", [128, prep["n_chunks"][cl] * D], CLS_DT[cl],
                       kind="ExternalInput").ap()
        for cl in range(2)
    ]
    wgt = nc.dram_tensor("wgt", [128, w_cols], FP16, kind="ExternalInput").ap()
    # out[p, t*D + d] holds entity row 128*t + p (fp16; host un-permutes and
    # upcasts - saves half the writeback DMA).
    out = nc.dram_tensor("out", [128, n_etiles * D], FP16, kind="ExternalOutput").ap()

    n_segs = [-(-prep["n_chunks"][cl] // seg_slots) for cl in range(2)]

    with tile.TileContext(nc) as tc, contextlib.ExitStack() as ctx:
        meta = ctx.enter_context(tc.tile_pool(name="meta", bufs=1))
        gats = [
            ctx.enter_context(
                tc.tile_pool(name=f"gat{cl}", bufs=n_segs[cl] + gat_slack))
            for cl in range(2)
        ]
        op = ctx.enter_context(tc.tile_pool(name="op", bufs=2))
        pp = ctx.enter_context(tc.tile_pool(name="pp", bufs=1, space="PSUM"))

        w_sb = meta.tile([128, w_cols], FP16)
        nc.sync.dma_start(w_sb[:], wgt[:])

        psums = [
            pp.tile([128, D], FP32, tag=f"ps{t}", name=f"ps{t}")
            for t in range(n_etiles)
        ]

        def body(rep):
            segs = [[], []]
            for cl in range(2):
                ns = 1 if mode == "pe_only" else n_segs[cl]
                for s in range(ns):
                    c0 = s * seg_slots
                    cols = min(seg_slots, prep["n_chunks"][cl] - c0)
                    g = gats[cl].tile([128, seg_slots * D], CLS_DT[cl],
                                      tag="g", name=f"g{cl}_{rep}_{s}")
                    # round-robin the DMA queues (SP/Act HWDGE + Pool SWDGE)
                    # so no single descriptor ring feed-limits the transfer
                    nseg = len(segs[0]) + len(segs[1])
                    eng = (nc.sync, nc.scalar, nc.gpsimd)[nseg % 3]
                    eng.dma_start(
                        g[:, : cols * D], encs[cl][:, c0 * D : (c0 + cols) * D]
                    )
                    segs[cl].append(g)
            if mode == "dma_only":
                jk = op.tile([128, 4], FP16, tag="junk", name=f"jk_{rep}")
                nc.vector.tensor_copy(jk[:], segs[0][-1][:, :4])
            else:
                for i, (cl, c, t, o, wa) in enumerate(mm_ops):
                    s, lc = divmod(c, seg_slots)
                    if mode == "pe_only":
                        s, lc = 0, lc % 4
                    nc.tensor.matmul(
                        out=psums[t][o : o + WIN, :],
                        lhsT=w_sb[:, wa : wa + WIN],
                        rhs=segs[cl][s][:, lc * D : (lc + 1) * D],
                        start=(first_of[(t, o)] == i),
                        stop=(last_of[(t, o)] == i),
                    )
            o_t = op.tile([128, n_etiles * D], FP16, tag="o", name=f"o_{rep}")
            for t in range(n_etiles):
                if mode == "full":
                    nc.vector.tensor_copy(o_t[:, t * D : (t + 1) * D], psums[t][:])
                else:
                    nc.vector.memset(o_t[:, t * D : (t + 1) * D], 0.0)
            nc.scalar.dma_start(out[:], o_t[:])

        for rep in range(n_reps):
            body(rep)

    split_excess_waits(nc)
    return nc


# ---------------------------------------------------------------------------
# Public entry point
# ---------------------------------------------------------------------------
KERNEL_CFG = dict(seg_slots=SEG_SLOTS)


def make_in_maps(prep, enc_np):
    tabs = build_tables(enc_np, prep)
    return [
        {
            "enc0": tabs[0][c],
            "enc1": tabs[1][c],
            "wgt": np.ascontiguousarray(prep["wgt_mat"][c].astype(np.float16)),
        }
        for c in range(N_CORES)
    ]


def kernel(enc_seq, info, num_entities):
    enc_np = np.ascontiguousarray(np.asarray(enc_seq, dtype=np.float32))
    prep = _host_prep(np.asarray(info), num_entities)
    nc = build_program(prep, n_reps=1, **KERNEL_CFG)
    in_maps = make_in_maps(prep, enc_np)
    r = run_bass_kernel_spmd(nc, in_maps, list(range(N_CORES)))

    E_ = prep["E"]
    n_etiles = prep["n_etiles"]
    entities = np.zeros((E_, D), dtype=np.float32)
    for c in range(N_CORES):
        ents = prep["ent_of_core"][c]
        o = r.results[c]["out"].astype(np.float32)
        o = o.reshape(128, n_etiles, D).transpose(1, 0, 2)
        entities[ents] = o.reshape(n_etiles * 128, D)[: len(ents)]
    return entities


# revision 29
# speedup vs baseline: 1.1627x; 1.1627x over previous
"""Trainium2 Bass kernel for segment_reduce (span mean-pool -> entity mean).

Strategy (8 NeuronCores, SPMD, one program + per-core data):
  - Entities are dealt round-robin across the 8 cores in sorted-size order
    ("snake deal"), so every core owns e_pc entities whose per-rank row
    counts nearly match across cores.  Each core owns all mentions of its
    entities, so no cross-core reduction is needed.
  - The host lays each core's span token-rows out in *streaming order*:
    rank-k entity occupies padded slot range [S[k], S[k+1]) identical on
    every core (quota = max rows over cores).  The table is uploaded as
    [128, SLOTS*D] where slot i = (chunk i//128, part i%128), so the device
    needs NO gather at all - just big sequential direct DMAs.
  - Rows are split into two precision classes: mentions shorter than
    LEN_SPLIT tokens (large 1/len weights) stay fp16; longer mentions are
    stored as float8e3 (e3m4).  Each class has its own slot schedule/table;
    both feed the same PSUM accumulators (measured rel err ~9e-3 vs fp32).
  - Because slots are entity-sorted, each 128-row chunk covers only a few
    consecutive entity ranks.  The span-sum AND entity-scatter fuse into a
    single narrow PSUM-accumulated matmul per chunk:
        psum[rank, :] += sum_p W[p, rank] * row[p, :]
    where W[p, col] (fp16, one 64-wide zero-padded block per matmul, base
    partition must be in {0,32,64}) is pure index data, built on the host
    and uploaded ONCE (not per iteration).
  - Per iteration the device does: ~12 direct DMAs (~5.8MB), ~183 narrow
    matmuls (PE), 4 psum->sbuf fp16 copies, 1 fp16 output DMA (host
    upcasts).  Vector/Pool engines are essentially idle; the kernel runs at
    the joint HBM-read / PE-stream roofline (~15us/iter vs 78us for the
    indirect-DMA + on-device-W baseline).
"""

import contextlib

import numpy as np

from concourse import bass, mybir
import concourse.tile as tile
from concourse.bass_utils import run_bass_kernel_spmd

# Problem constants (nn_BaseModel_69355131896059)
T, D, M, E, L_MAX = 200000, 256, 20000, 4000, 16
N_CORES = 8
FP32 = mybir.dt.float32
FP16 = mybir.dt.float16
FP8E3 = mybir.dt.float8e3

LEN_SPLIT = 4   # mentions shorter than this stay fp16; longer go fp8 (e3m4)
SEG_SLOTS = 32  # 128-row chunks per DMA segment
WIN = 64        # psum window width (base partition must be in {0,32,64})


def _np_dt(dt):
    return mybir.dt.np(dt)


# ---------------------------------------------------------------------------
# Walrus in this container rejects instructions carrying more than ~2 sync
# commands ("Too many sync wait commands").  After Tile scheduling, split
# excess sem waits onto same-engine NOPs inserted before the instruction.
# ---------------------------------------------------------------------------
_WAIT_LIMIT = 1
_nsplit = [0]


def split_excess_waits(nc, limit=_WAIT_LIMIT):
    for fn in nc.m.functions:
        for bb in fn.blocks:
            insts = list(bb.instructions)
            if not any(
                i.sync_info is not None
                and i.sync_info.on_wait
                and len(i.sync_info.on_wait) > limit
                for i in insts
            ):
                continue
            out = []
            for inst in insts:
                si = inst.sync_info
                if si is not None and si.on_wait and len(si.on_wait) > limit:
                    waits = list(si.on_wait)
                    keep, extra = waits[-limit:], waits[:-limit]
                    for s in range(0, len(extra), limit):
                        nop = mybir.InstNoOp(
                            name=f"waitsplit-{_nsplit[0]}",
                            engine=inst.engine,
                            sync_info=mybir.SyncInfo(
                                on_wait=extra[s : s + limit], on_update=[]
                            ),
                        )
                        _nsplit[0] += 1
                        out.append(nop)
                    inst.sync_info = mybir.SyncInfo(
                        on_wait=keep, on_update=list(si.on_update or [])
                    )
                out.append(inst)
            bb.instructions = out


# ---------------------------------------------------------------------------
# Host-side prep: entity->core snake deal, per-class slot schedules, W
# matrix, tables.  All of it is index bookkeeping + byte movement; the
# arithmetic (sums, weighting) happens on device.
# ---------------------------------------------------------------------------
def _host_prep(info, num_entities):
    E_ = int(num_entities)
    eid = np.asarray(info[:, 0], dtype=np.int64)
    starts = np.asarray(info[:, 2], dtype=np.int64)
    ends = np.asarray(info[:, 3], dtype=np.int64)
    lens = ends - starts
    glen = np.minimum(lens, L_MAX)  # reference only pools the first L_MAX rows
    cnt = np.bincount(eid, minlength=E_).astype(np.float64)
    w_all = 1.0 / (np.maximum(lens, 1) * np.maximum(cnt[eid], 1.0))

    cls_of_m = (lens >= LEN_SPLIT).astype(np.int64)  # 0 = fp16, 1 = fp8
    n_cls = 2
    bytes_of_cls = np.array([2, 1])

    # Deal entities to cores: sort by fp8 rows (the dominant class) so each
    # rank group of 8 has near-equal fp8 quota, then re-sort within blocks of
    # 128 by fp16 rows to also equalize the fp16 quota (r8 spread within a
    # block is tiny, so this costs the fp8 class almost nothing).
    ent_r = np.zeros((E_, n_cls), dtype=np.int64)
    np.add.at(ent_r, (eid, cls_of_m), glen)
    e_pc = -(-E_ // N_CORES)
    order = np.argsort(-ent_r[:, 1], kind="stable")
    BLK = 128
    for i in range(0, E_, BLK):
        blk = order[i : i + BLK]
        order[i : i + BLK] = blk[np.argsort(-ent_r[blk, 0], kind="stable")]
    order_pad = np.concatenate([order, np.full(e_pc * N_CORES - E_, -1, np.int64)])
    groups = order_pad.reshape(e_pc, N_CORES)  # [rank, core] -> entity (-1 pad)
    rank_of_ent = np.zeros(E_, dtype=np.int64)
    core_of_ent = np.zeros(E_, dtype=np.int64)
    rk, ck = np.nonzero(groups >= 0)
    rank_of_ent[groups[rk, ck]] = rk
    core_of_ent[groups[rk, ck]] = ck
    m_rank = rank_of_ent[eid]
    m_core = core_of_ent[eid]

    n_etiles = -(-e_pc // 128)
    classes = []
    n_ops_so_far = 0
    mm_ops = []  # (class, chunk, psum_tile, psum_part_off, wcol_a)
    for cl in range(n_cls):
        msel = cls_of_m == cl
        # per-(rank, core) class rows -> quota
        per_rc = np.zeros((e_pc, N_CORES), dtype=np.int64)
        np.add.at(per_rc, (m_rank[msel], m_core[msel]), glen[msel])
        quota = per_rc.max(axis=1)
        S = np.concatenate([[0], np.cumsum(quota)])
        n_slots = int(S[-1])
        n_chunks = max(1, -(-n_slots // 128))
        padded = n_chunks * 128

        c_start = np.arange(n_chunks) * 128
        l0 = np.clip(np.searchsorted(S, c_start, side="right") - 1, 0, e_pc - 1)
        l1 = np.minimum(np.searchsorted(S, c_start + 128, side="left"), e_pc)
        l1 = np.maximum(l1, l0 + 1)
        a0 = l0 // WIN
        a1 = (l1 - 1) // WIN
        op_base = n_ops_so_far + np.concatenate(
            [[0], np.cumsum(a1 - a0 + 1)]
        ).astype(np.int64)
        for c in range(n_chunks):
            for a in range(int(a0[c]), int(a1[c]) + 1):
                t = (WIN * a) // 128
                mm_ops.append((cl, c, t, WIN * a - 128 * t, WIN * len(mm_ops)))
        n_ops_so_far = len(mm_ops)

        rank_of_slot = np.clip(
            np.searchsorted(S, np.arange(padded), side="right") - 1, 0, e_pc - 1
        )
        chunk_of_slot = np.arange(padded) // 128
        wcol_of_slot = (
            WIN * (op_base[chunk_of_slot] + rank_of_slot // WIN - a0[chunk_of_slot])
            + rank_of_slot % WIN
        )
        classes.append(dict(
            msel=msel, S=S, n_chunks=n_chunks, padded=padded,
            wcol_of_slot=wcol_of_slot,
        ))

    w_cols = WIN * len(mm_ops)
    first_of, last_of = {}, {}
    for i, (cl, c, t, o, wa) in enumerate(mm_ops):
        first_of.setdefault((t, o), i)
        last_of[(t, o)] = i

    # per-core slot -> (enc row | -1, weight); shared-structure W matrix
    wgt_mat = np.zeros((N_CORES, 128, w_cols), dtype=np.float32)
    rows_of_slot = []
    for cl in range(n_cls):
        k = classes[cl]
        row_of_slot = np.full((N_CORES, k["padded"]), -1, dtype=np.int64)
        for c in range(N_CORES):
            sel = np.nonzero((m_core == c) & k["msel"])[0]
            sel = sel[np.argsort(m_rank[sel], kind="stable")]
            g = glen[sel]
            cum = np.cumsum(g) - g
            rank_base = np.cumsum(np.bincount(
                m_rank[sel], weights=g.astype(np.float64), minlength=e_pc))
            rank_base = np.concatenate([[0], rank_base])[:-1].astype(np.int64)
            base = k["S"][m_rank[sel]] + (cum - rank_base[m_rank[sel]])
            within = np.arange(int(g.sum())) - np.repeat(cum, g)
            tok_slot = (np.repeat(base, g) + within).astype(np.int64)
            tok_row = np.repeat(starts[sel], g) + within
            tok_w = np.repeat(w_all[sel], g)
            row_of_slot[c, tok_slot] = tok_row
            wgt_mat[c, tok_slot % 128, k["wcol_of_slot"][tok_slot]] = tok_w
        rows_of_slot.append(row_of_slot)

    ent_of_core = [groups[:, c][groups[:, c] >= 0] for c in range(N_CORES)]
    return {
        "n_chunks": [classes[cl]["n_chunks"] for cl in range(n_cls)],
        "n_etiles": n_etiles,
        "w_cols": w_cols,
        "mm_ops": mm_ops,
        "first_of": first_of,
        "last_of": last_of,
        "rows_of_slot": rows_of_slot,
        "wgt_mat": wgt_mat,
        "ent_of_core": ent_of_core,
        "e_pc": e_pc,
        "E": E_,
    }


CLS_DT = [FP16, FP8E3]


def build_tables(enc_np, prep):
    """Per-core, per-class streaming-order tables [128, SLOTS*D]."""
    tabs = [[] for _ in range(2)]
    for cl in range(2):
        n_chunks = prep["n_chunks"][cl]
        padded = n_chunks * 128
        ndt = _np_dt(CLS_DT[cl])
        for c in range(N_CORES):
            rows = prep["rows_of_slot"][cl][c]
            tab = np.zeros((padded, D), dtype=ndt)
            v = rows >= 0
            tab[v] = enc_np[rows[v]].astype(ndt)
            tab = np.ascontiguousarray(
                tab.reshape(n_chunks, 128, D).transpose(1, 0, 2)
                .reshape(128, n_chunks * D)
            )
            tabs[cl].append(tab)
    return tabs


# ---------------------------------------------------------------------------
# Device program
# ---------------------------------------------------------------------------
def build_program(prep, n_reps=1, mode="full", seg_slots=SEG_SLOTS, gat_slack=2):
    n_etiles = prep["n_etiles"]
    w_cols = prep["w_cols"]
    mm_ops = prep["mm_ops"]
    first_of = prep["first_of"]
    last_of = prep["last_of"]

    nc = bass.Bass("TRN2", target_bir_lowering=False, debug=False,
                   num_devices=N_CORES)
    encs = [
        nc.dram_tensor(f"enc{cl}", [128, prep["n_chunks"][cl] * D], CLS_DT[cl],
                       kind="ExternalInput").ap()
        for cl in range(2)
    ]
    wgt = nc.dram_tensor("wgt", [128, w_cols], FP16, kind="ExternalInput").ap()
    # out[p, t*D + d] holds entity row 128*t + p (fp16; host un-permutes and
    # upcasts - saves half the writeback DMA).
    out = nc.dram_tensor("out", [128, n_etiles * D], FP16, kind="ExternalOutput").ap()

    n_segs = [-(-prep["n_chunks"][cl] // seg_slots) for cl in range(2)]

    with tile.TileContext(nc) as tc, contextlib.ExitStack() as ctx:
        meta = ctx.enter_context(tc.tile_pool(name="meta", bufs=1))
        gats = [
            ctx.enter_context(
                tc.tile_pool(name=f"gat{cl}", bufs=n_segs[cl] + gat_slack))
            for cl in range(2)
        ]
        op = ctx.enter_context(tc.tile_pool(name="op", bufs=2))
        pp = ctx.enter_context(tc.tile_pool(name="pp", bufs=1, space="PSUM"))

        w_sb = meta.tile([128, w_cols], FP16)
        nc.sync.dma_start(w_sb[:], wgt[:])

        psums = [
            pp.tile([128, D], FP32, tag=f"ps{t}", name=f"ps{t}")
            for t in range(n_etiles)
        ]

        def body(rep):
            segs = [[], []]
            for cl in range(2):
                ns = 1 if mode == "pe_only" else n_segs[cl]
                for s in range(ns):
                    c0 = s * seg_slots
                    cols = min(seg_slots, prep["n_chunks"][cl] - c0)
                    g = gats[cl].tile([128, seg_slots * D], CLS_DT[cl],
                                      tag="g", name=f"g{cl}_{rep}_{s}")
                    # alternate the two HWDGE queues (SP, Activation) so a
                    # single sequencer ring never feed-limits the transfer;
                    # gpsimd's SWDGE ring measured slower (994ns/inst desc-gen)
                    eng = nc.sync if (len(segs[0]) + len(segs[1])) % 2 == 0 else nc.scalar
                    eng.dma_start(
                        g[:, : cols * D], encs[cl][:, c0 * D : (c0 + cols) * D]
                    )
                    segs[cl].append(g)
            if mode == "dma_only":
                jk = op.tile([128, 4], FP16, tag="junk", name=f"jk_{rep}")
                nc.vector.tensor_copy(jk[:], segs[0][-1][:, :4])
            else:
                for i, (cl, c, t, o, wa) in enumerate(mm_ops):
                    s, lc = divmod(c, seg_slots)
                    if mode == "pe_only":
                        s, lc = 0, lc % 4
                    nc.tensor.matmul(
                        out=psums[t][o : o + WIN, :],
                        lhsT=w_sb[:, wa : wa + WIN],
                        rhs=segs[cl][s][:, lc * D : (lc + 1) * D],
                        start=(first_of[(t, o)] == i),
                        stop=(last_of[(t, o)] == i),
                    )
            o_t = op.tile([128, n_etiles * D], FP16, tag="o", name=f"o_{rep}")
            for t in range(n_etiles):
                if mode == "full":
                    nc.vector.tensor_copy(o_t[:, t * D : (t + 1) * D], psums[t][:])
                else:
                    nc.vector.memset(o_t[:, t * D : (t + 1) * D], 0.0)
            nc.scalar.dma_start(out[:], o_t[:])

        for rep in range(n_reps):
            body(rep)

    split_excess_waits(nc)
    return nc


# ---------------------------------------------------------------------------
# Public entry point
# ---------------------------------------------------------------------------
KERNEL_CFG = dict(seg_slots=SEG_SLOTS)


def make_in_maps(prep, enc_np):
    tabs = build_tables(enc_np, prep)
    return [
        {
            "enc0": tabs[0][c],
            "enc1": tabs[1][c],
            "wgt": np.ascontiguousarray(prep["wgt_mat"][c].astype(np.float16)),
        }
        for c in range(N_CORES)
    ]


def kernel(enc_seq, info, num_entities):
    enc_np = np.ascontiguousarray(np.asarray(enc_seq, dtype=np.float32))
    prep = _host_prep(np.asarray(info), num_entities)
    nc = build_program(prep, n_reps=1, **KERNEL_CFG)
    in_maps = make_in_maps(prep, enc_np)
    r = run_bass_kernel_spmd(nc, in_maps, list(range(N_CORES)))

    E_ = prep["E"]
    n_etiles = prep["n_etiles"]
    entities = np.zeros((E_, D), dtype=np.float32)
    for c in range(N_CORES):
        ents = prep["ent_of_core"][c]
        o = r.results[c]["out"].astype(np.float32)
        o = o.reshape(128, n_etiles, D).transpose(1, 0, 2)
        entities[ents] = o.reshape(n_etiles * 128, D)[: len(ents)]
    return entities
